# revision 15
# baseline (speedup 1.0000x reference)
"""Trainium2 Bass kernel for nn_MDCR (multi-dilated conv residual block).

Pipeline per batch image (one NeuronCore per batch element, 8 total):
  stage 1: four depthwise 3x3 dilated convs (rates 1/6/12/18, 128 ch each)
           -> +bias -> BN(eval) -> ReLU
  stage 2: shared 1x1 "mix" over the 4 branch outputs (4->4 per channel)
           -> BN -> ReLU
  stage 3: dense 1x1 conv 512->512 -> BN -> ReLU

Mapping (v2):
  - Data-parallel over batch: core b handles x[b] fully.
  - fp16 everywhere on device; x is cast+padded on the host, output DMA'd
    as fp16 and cast back on the host.
  - Branch r=1 runs on the PE as diagonal 128x128 tap matmuls (its odd
    W-offsets would break DVE packed modes; PE reads are offset-agnostic).
  - Branches r=6/12/18 run on the DVE: per-tap tensor_scalar_mul (4x_2p
    packed mode, 4 elem/cycle/lane) into a tmp tile, then in-place
    tensor_tensor add (2x_1p, 2 elem/cycle) into an fp16 accumulator.
    All W offsets are even for even rates, keeping 4B alignment.
  - DVE-branch epilogue: one tensor_scalar (add bias, max 0) at 4x.
  - Stage 2: 16 scaled-identity matmuls per 512-px chunk on PE.
  - Stage 3: dense 512x512 as 4x4 blocks of 128x128 on PE.
  - s2/s3/r1 epilogues (PSUM -> SBUF + bias + ReLU) on the ACT engine.
  - BN scales folded into conv/mix/s3 weights on the host; biases only
    in the epilogues.
"""

import ml_dtypes
import numpy as np

import concourse.bass as bass
import concourse.mybir as mybir
import concourse.tile as tile
from concourse.bass_utils import run_bass_kernel_spmd
from concourse.vector_clock import ScopedClock


def _patched_drain_and_barrier(self, tick_clock, wait_clock):
    """This walrus build rejects sync waits on the Drain opcode (CTRL
    NO_STRUCT encoding). Split the kernel-tail drain's aggregated sem waits
    onto individual sync-engine NoOps, then emit a bare drain."""
    nc = self.nc
    collector = nc.sync.nop(nofuse=True, hint="tail_wait_collector")
    wait_clock.add_sem_waits(
        collector.ins, ScopedClock({None: tick_clock.global_clock}))
    si = collector.ins.sync_info
    waits = list(si.on_wait) if si is not None else []
    if len(waits) > 1:
        collector.ins.sync_info = mybir.SyncInfo(
            on_wait=[waits[0]], on_update=list(si.on_update))
        for w in waits[1:]:
            n = nc.sync.nop(nofuse=True, hint="tail_wait")
            n.ins.sync_info = mybir.SyncInfo(on_wait=[w], on_update=[])
    nc.sync.drain()
    nc.all_engine_barrier()
    assert self.sems is not None
    popped = nc._tile_sem_poison_stack.pop()
    assert popped is self._sem_poison
    nc.clear_and_free_semaphores(list(self.sems.allocated().values()))
    nc.all_engine_barrier()


tile.TileContext._drain_and_barrier = _patched_drain_and_barrier


def _split_multi_waits(nc):
    """This walrus build supports at most one sync-wait per instruction.
    Move extra waits onto same-engine NoOps placed immediately before."""
    for fn in nc.m.functions:
        for blk in fn.blocks:
            insts = blk.instructions
            if not any(i.sync_info and len(i.sync_info.on_wait) > 1
                       for i in insts):
                continue
            out = []
            for ins in insts:
                si = ins.sync_info
                if si is not None and len(si.on_wait) > 1:
                    waits = list(si.on_wait)
                    for w in waits[:-1]:
                        nop = mybir.InstNoOp(
                            name=nc.get_next_instruction_name(),
                            sync_info=mybir.SyncInfo(on_wait=[w], on_update=[]),
                            bass_nofuse=True,
                            engine=ins.engine,
                        )
                        try:
                            nc.register_instruction(nop, overwrite=True)
                        except Exception:
                            pass
                        out.append(nop)
                    ins.sync_info = mybir.SyncInfo(
                        on_wait=[waits[-1]], on_update=list(si.on_update))
                out.append(ins)
            blk.instructions = out


EPS = 1e-5
RATES = (1, 6, 12, 18)
PADS = (2, 6, 12, 18)       # W padding per branch (r=1 padded to 2: even base)
B, C, H, W = 8, 512, 96, 96
CQ = C // 4                 # 128, one partition chunk per branch
PIX = H * W
STRIP = 16                  # rows per pipeline strip
SNT = STRIP * W             # 1536 px per strip
N_STRIPS = H // STRIP       # 6
GROUP = 4                   # r1 PE conv: rows per psum group (384 px <= 512)
CHUNK = 512                 # px per stage-2/3 psum bank chunk
F16 = mybir.dt.float16
F32 = mybir.dt.float32

_PROG_CACHE = {}

# Engine assignment knobs for the DVE-side branches (branch 0 is PE-only):
# ACT_TAPS: (i, t) whose tap product runs on ACT (scale-AP copy) not DVE.
# POOL_TAPS: per-branch taps accumulated on a Pool-side chain; the FIRST
#   listed tap must be clamp-free (dh=0, i.e. t in {3,5}) so the pool
#   accumulator covers the full strip before sub-range adds land on it.
ACT_TAPS = set()
POOL_TAPS = {2: (5, 0, 2), 3: (5, 0, 2)}
STORE_ENGINE = "scalar"   # which engine's DMA ring carries output stores
                          # (gpsimd DMA hits "ISA wrong length" inside For_i)
CONSTS_IN_BODY = False    # reload weights every loop iteration (debug)


def _np_f16(a):
    return np.asarray(a, dtype=np.float32).astype(np.float16)


def _host_consts(wdw, bdw, gdw, bedw, mdw, vdw, ws, bs, gs, bes, ms, vs,
                 wo, bo, go, beo, mo, vo):
    """Fold BN constants into weights and build PE weight blocks."""
    f64 = np.float64
    # stage 1: y = relu(conv(x; w*s1) + b1)
    inv1 = np.asarray(gdw, f64) / np.sqrt(np.asarray(vdw, f64) + EPS)  # [4,128]
    b1 = (np.asarray(bdw, f64) - np.asarray(mdw, f64)) * inv1 + np.asarray(bedw, f64)
    wdw_s = np.asarray(wdw, f64) * inv1[:, :, None, None, None]        # [4,128,1,3,3]

    # stage 2: z_o = relu(sum_i Amix[o,i]*y_i + b2[o])
    invs = np.asarray(gs, f64) / np.sqrt(np.asarray(vs, f64) + EPS)    # [4]
    W4 = np.asarray(ws, f64)[:, :, 0, 0]                               # [o,i]
    Amix = W4 * invs[:, None]
    b2 = (np.asarray(bs, f64) - np.asarray(ms, f64)) * invs + np.asarray(bes, f64)

    # stage 3: out = relu(Wo' z + b3), Wo' = diag(s3) Wo
    inv3 = np.asarray(go, f64) / np.sqrt(np.asarray(vo, f64) + EPS)    # [512]
    Wo = np.asarray(wo, f64)[:, :, 0, 0]                               # [512,512]
    Wo_s = Wo * inv3[:, None]
    b3 = (np.asarray(bo, f64) - np.asarray(mo, f64)) * inv3 + np.asarray(beo, f64)

    consts = {}
    # r1 diag blocks: [128(k=c), 9(tap), 128(m=c)], s1-folded
    dw = np.zeros((CQ, 9, CQ), np.float32)
    for t in range(9):
        np.fill_diagonal(dw[:, t, :], wdw_s[0, :, 0, t // 3, t % 3].astype(np.float32))
    consts["dww"] = _np_f16(dw)

    # mix blocks: [128(k=c), 16(o*4+i), 128(m=c)] = Amix[o,i] * I
    mixw = np.zeros((CQ, 16, CQ), np.float32)
    for o in range(4):
        for i in range(4):
            np.fill_diagonal(mixw[:, o * 4 + i, :], np.float32(Amix[o, i]))
    consts["mixw"] = _np_f16(mixw)

    # stage-3 blocks: [128(k=c of z-chunk o), 16(m*4+o), 128(mc)]
    # z-chunk o, row c  <->  original z channel 4c+o
    s3w = np.zeros((CQ, 16, CQ), np.float32)
    for m in range(4):
        blk = Wo_s[128 * m:128 * (m + 1), :].astype(np.float32)  # [mc, 512]
        for o in range(4):
            s3w[:, m * 4 + o, :] = blk[:, o::4].T  # [c, mc]
    consts["s3w"] = _np_f16(s3w)

    # per-channel tap weights (s1-folded) for DVE/ACT: [128, 36], col i*9+t
    dws = np.zeros((CQ, 36), np.float32)
    for i in range(4):
        for t in range(9):
            dws[:, i * 9 + t] = wdw_s[i, :, 0, t // 3, t % 3].astype(np.float32)
    consts["dws"] = dws

    consts["b1"] = np.ascontiguousarray(np.asarray(b1, np.float32).T)  # [128,4]
    consts["b2"] = np.ascontiguousarray(
        np.broadcast_to(np.asarray(b2, np.float32)[None, :], (CQ, 4))).copy()
    consts["b3"] = np.ascontiguousarray(
        np.asarray(b3, np.float32).reshape(4, CQ).T)  # [128,4] col m
    return consts


def _tap_list(r, h0, h1):
    """Valid taps for output rows [h0, h1); center tap first (always full)."""
    taps = []
    for t in range(9):
        dh, dw = t // 3 - 1, t % 3 - 1
        lo = max(h0, -dh * r)
        hi = min(h1, H - dh * r)
        if lo < hi:
            taps.append((t, dh, dw, lo, hi))
    taps.sort(key=lambda e: (e[3] != h0 or e[4] != h1,))
    return taps


def _build_program(loop_n=None):
    nc = bass.Bass("TRN2", target_bir_lowering=False, debug=False, num_devices=8)

    xp_d = [nc.dram_tensor(f"xp{i}", [CQ, H, W + 2 * PADS[i]], F16,
                           kind="ExternalInput") for i in range(4)]
    dww_d = nc.dram_tensor("dww", [CQ, 9, CQ], F16, kind="ExternalInput")
    mixw_d = nc.dram_tensor("mixw", [CQ, 16, CQ], F16, kind="ExternalInput")
    s3w_d = nc.dram_tensor("s3w", [CQ, 16, CQ], F16, kind="ExternalInput")
    dws_d = nc.dram_tensor("dws", [CQ, 36], F32, kind="ExternalInput")
    b1_d = nc.dram_tensor("b1", [CQ, 4], F32, kind="ExternalInput")
    b2_d = nc.dram_tensor("b2", [CQ, 4], F32, kind="ExternalInput")
    b3_d = nc.dram_tensor("b3", [CQ, 4], F32, kind="ExternalInput")
    out_d = nc.dram_tensor("out", [C, PIX], F16, kind="ExternalOutput")

    with tile.TileContext(nc) as tc:
        with (
            tc.tile_pool(name="consts", bufs=1) as cpool,
            tc.tile_pool(name="xpad", bufs=1) as xpool,
            tc.tile_pool(name="ys", bufs=2) as ypool,
            tc.tile_pool(name="accs", bufs=2) as apool,
            tc.tile_pool(name="tmps", bufs=4) as tpool,
            tc.tile_pool(name="zs", bufs=2) as zpool,
            tc.tile_pool(name="outs", bufs=2) as opool,
            tc.tile_pool(name="ps1", bufs=2, space=bass.MemorySpace.PSUM) as ps1,
            tc.tile_pool(name="ps2", bufs=3, space=bass.MemorySpace.PSUM) as ps2,
            tc.tile_pool(name="ps3", bufs=3, space=bass.MemorySpace.PSUM) as ps3,
        ):
          # ---- constants to SBUF once, outside the timing loop
          dww = cpool.tile([CQ, 9, CQ], F16, tag="dww")
          nc.sync.dma_start(dww[:], dww_d[:])
          mixw = cpool.tile([CQ, 16, CQ], F16, tag="mixw")
          nc.sync.dma_start(mixw[:], mixw_d[:])
          s3w = cpool.tile([CQ, 16, CQ], F16, tag="s3w")
          nc.sync.dma_start(s3w[:], s3w_d[:])
          dws = cpool.tile([CQ, 36], F32, tag="dws")
          nc.sync.dma_start(dws[:], dws_d[:])
          b1 = cpool.tile([CQ, 4], F32, tag="b1")
          nc.sync.dma_start(b1[:], b1_d[:])
          b2 = cpool.tile([CQ, 4], F32, tag="b2")
          nc.sync.dma_start(b2[:], b2_d[:])
          b3 = cpool.tile([CQ, 4], F32, tag="b3")
          nc.sync.dma_start(b3[:], b3_d[:])

          xpad = []
          for i in range(4):
              wp = W + 2 * PADS[i]
              t = xpool.tile([CQ, H, wp], F16, tag=f"xpad{i}")
              xpad.append(t)

          def _body():
            # ---- padded x tiles (fp16, pre-padded on host)
            for i in range(4):
                for s in range(N_STRIPS):
                    nc.sync.dma_start(
                        xpad[i][:, s * STRIP:(s + 1) * STRIP, :],
                        xp_d[i][:, s * STRIP:(s + 1) * STRIP, :])

            def _r1_conv(s):
                # r1 branch on PE: diag matmuls per 4-row group
                h0 = s * STRIP
                pad0 = PADS[0]
                y0 = ypool.tile([CQ, SNT], F16, tag="y0")
                for g in range(STRIP // GROUP):
                    g0, g1 = h0 + g * GROUP, h0 + (g + 1) * GROUP
                    taps = _tap_list(1, g0, g1)
                    p1 = ps1.tile([CQ, CHUNK], F32, tag="p1")
                    for j, (t, dh, dw, lo, hi) in enumerate(taps):
                        rhs = xpad[0][:, lo + dh:hi + dh,
                                      pad0 + dw:pad0 + dw + W]
                        nc.tensor.matmul(
                            p1[:, (lo - g0) * W:(hi - g0) * W],
                            dww[:, t, :], rhs,
                            start=(j == 0), stop=(j == len(taps) - 1))
                    nc.scalar.activation(
                        y0[:, (g0 - h0) * W:(g1 - h0) * W],
                        p1[:, 0:GROUP * W],
                        mybir.ActivationFunctionType.Relu,
                        bias=b1[:, 0:1], scale=1.0)
                return y0

            # ---- per-strip pipeline; r1 conv runs one strip ahead so the
            # PE has independent work queued while stage 2 waits on DVE ys
            y0_next = _r1_conv(0)
            for s in range(N_STRIPS):
                h0, h1 = s * STRIP, (s + 1) * STRIP
                ys = [y0_next]

                # r6/r12/r18 branches on DVE (+ optional ACT/Pool offloads).
                # Two accumulator chains: `acc` on DVE, `accp` on Pool; the
                # pool chain is merged into acc at the end on DVE.
                for i in (1, 2, 3):
                    r, pad = RATES[i], PADS[i]
                    pool_taps = list(POOL_TAPS.get(i, ()))
                    acc = apool.tile([CQ, SNT], F16, tag=f"acc{i}")
                    if pool_taps:
                        accp = apool.tile([CQ, SNT], F16, tag=f"accp{i}",
                                          name=f"accp{i}")
                    else:
                        accp = None
                    dve_started = pool_started = False
                    for t, dh, dw, lo, hi in _tap_list(r, h0, h1):
                        xin = xpad[i][:, lo + dh * r:hi + dh * r,
                                      pad + dw * r:pad + dw * r + W]
                        wsc = dws[:, i * 9 + t:i * 9 + t + 1]
                        on_pool = t in pool_taps
                        tgt = accp if on_pool else acc
                        sub = tgt[:, (lo - h0) * W:(hi - h0) * W]
                        first = pool_started if on_pool else dve_started
                        if not first:
                            # init the chain with this tap's product
                            nc.vector.tensor_scalar_mul(sub, xin, wsc)
                            if on_pool:
                                pool_started = True
                            else:
                                dve_started = True
                            continue
                        tmp = tpool.tile([CQ, SNT], F16, tag="tmp")
                        tsub = tmp[:, 0:(hi - lo) * W]
                        if (i, t) in ACT_TAPS:
                            nc.scalar.activation(
                                tsub, xin,
                                mybir.ActivationFunctionType.Identity,
                                bias=0.0, scale=wsc)
                        else:
                            nc.vector.tensor_scalar_mul(tsub, xin, wsc)
                        eng = nc.gpsimd if on_pool else nc.vector
                        eng.tensor_tensor(
                            sub, sub, tsub, mybir.AluOpType.add)
                    if pool_taps:
                        nc.vector.tensor_tensor(
                            acc[:], acc[:], accp[:], mybir.AluOpType.add)
                    # epilogue: y = max(acc + b1, 0) at 4x on DVE
                    yb = ypool.tile([CQ, SNT], F16, tag=f"y{i}")
                    nc.vector.tensor_scalar(
                        yb[:], acc[:], b1[:, i:i + 1], 0.0,
                        mybir.AluOpType.add, mybir.AluOpType.max)
                    ys.append(yb)

                if s + 1 < N_STRIPS:
                    y0_next = _r1_conv(s + 1)

                # stage 2: mix on PE, 4 scaled-identity matmuls per (o, chunk)
                zs = []
                for o in range(4):
                    z = zpool.tile([CQ, SNT], F16, tag=f"z{o}")
                    for c in range(SNT // CHUNK):
                        p2 = ps2.tile([CQ, CHUNK], F32, tag="p2")
                        for i in range(4):
                            nc.tensor.matmul(
                                p2[:], mixw[:, o * 4 + i, :],
                                ys[i][:, c * CHUNK:(c + 1) * CHUNK],
                                start=(i == 0), stop=(i == 3))
                        nc.scalar.activation(
                            z[:, c * CHUNK:(c + 1) * CHUNK], p2[:],
                            mybir.ActivationFunctionType.Relu,
                            bias=b2[:, o:o + 1], scale=1.0)
                    zs.append(z)

                # stage 3: dense 512x512 on PE
                for m in range(4):
                    ot = opool.tile([CQ, SNT], F16, tag="ot")
                    for c in range(SNT // CHUNK):
                        p3 = ps3.tile([CQ, CHUNK], F32, tag="p3")
                        for o in range(4):
                            nc.tensor.matmul(
                                p3[:], s3w[:, m * 4 + o, :],
                                zs[o][:, c * CHUNK:(c + 1) * CHUNK],
                                start=(o == 0), stop=(o == 3))
                        nc.scalar.activation(
                            ot[:, c * CHUNK:(c + 1) * CHUNK], p3[:],
                            mybir.ActivationFunctionType.Relu,
                            bias=b3[:, m:m + 1], scale=1.0)
                    # stores on a separate DMA ring: keeps the sync ring
                    # loads-only so iter k+1 loads don't queue behind stores
                    store_eng = getattr(nc, STORE_ENGINE)
                    store_eng.dma_start(
                        out_d[CQ * m:CQ * (m + 1), h0 * W:h0 * W + SNT], ot[:])

          if loop_n:
              with tc.For_i(0, loop_n, 1):
                  _body()
          else:
              _body()
    _split_multi_waits(nc)
    return nc


def _get_program(loop_n=None):
    key = ("nc", loop_n)
    if key not in _PROG_CACHE:
        _PROG_CACHE[key] = _build_program(loop_n)
    return _PROG_CACHE[key]


def _in_maps(x, consts):
    x16 = np.asarray(x, np.float32).astype(np.float16).reshape(B, 4, CQ, H, W)
    maps = []
    for b in range(B):
        m = dict(consts)
        for i in range(4):
            p = PADS[i]
            m[f"xp{i}"] = np.ascontiguousarray(
                np.pad(x16[b, i], ((0, 0), (0, 0), (p, p))))
        maps.append(m)
    return maps


def run(x, consts, trace=False, loop_n=None):
    nc = _get_program(loop_n)
    res = run_bass_kernel_spmd(nc, _in_maps(x, consts), list(range(B)),
                               trace=trace)
    out = np.stack([res.results[b]["out"].reshape(C, H, W) for b in range(B)])
    return out.astype(np.float32), res


def kernel(x, wdw, bdw, gdw, bedw, mdw, vdw, ws, bs, gs, bes, ms, vs,
           wo, bo, go, beo, mo, vo):
    consts = _host_consts(wdw, bdw, gdw, bedw, mdw, vdw, ws, bs, gs, bes,
                          ms, vs, wo, bo, go, beo, mo, vo)
    out, _ = run(x, consts, trace=False)
    return out


# revision 17
# speedup vs baseline: 1.0006x; 1.0006x over previous
"""Trainium2 Bass kernel for nn_MDCR (multi-dilated conv residual block).

Pipeline per batch image (one NeuronCore per batch element, 8 total):
  stage 1: four depthwise 3x3 dilated convs (rates 1/6/12/18, 128 ch each)
           -> +bias -> BN(eval) -> ReLU
  stage 2: shared 1x1 "mix" over the 4 branch outputs (4->4 per channel)
           -> BN -> ReLU
  stage 3: dense 1x1 conv 512->512 -> BN -> ReLU

Mapping (v2):
  - Data-parallel over batch: core b handles x[b] fully.
  - fp16 everywhere on device; x is cast+padded on the host, output DMA'd
    as fp16 and cast back on the host.
  - Branch r=1 runs on the PE as diagonal 128x128 tap matmuls (its odd
    W-offsets would break DVE packed modes; PE reads are offset-agnostic).
  - Branches r=6/12/18 run on the DVE: per-tap tensor_scalar_mul (4x_2p
    packed mode, 4 elem/cycle/lane) into a tmp tile, then in-place
    tensor_tensor add (2x_1p, 2 elem/cycle) into an fp16 accumulator.
    All W offsets are even for even rates, keeping 4B alignment.
  - DVE-branch epilogue: one tensor_scalar (add bias, max 0) at 4x.
  - Stage 2: 16 scaled-identity matmuls per 512-px chunk on PE.
  - Stage 3: dense 512x512 as 4x4 blocks of 128x128 on PE.
  - s2/s3/r1 epilogues (PSUM -> SBUF + bias + ReLU) on the ACT engine.
  - BN scales folded into conv/mix/s3 weights on the host; biases only
    in the epilogues.
"""

import math

import ml_dtypes
import numpy as np

import concourse.bass as bass
import concourse.mybir as mybir
import concourse.tile as tile
from concourse.bass_utils import run_bass_kernel_spmd
from concourse.vector_clock import ScopedClock


def _patched_drain_and_barrier(self, tick_clock, wait_clock):
    """This walrus build rejects sync waits on the Drain opcode (CTRL
    NO_STRUCT encoding). Split the kernel-tail drain's aggregated sem waits
    onto individual sync-engine NoOps, then emit a bare drain."""
    nc = self.nc
    collector = nc.sync.nop(nofuse=True, hint="tail_wait_collector")
    wait_clock.add_sem_waits(
        collector.ins, ScopedClock({None: tick_clock.global_clock}))
    si = collector.ins.sync_info
    waits = list(si.on_wait) if si is not None else []
    if len(waits) > 1:
        collector.ins.sync_info = mybir.SyncInfo(
            on_wait=[waits[0]], on_update=list(si.on_update))
        for w in waits[1:]:
            n = nc.sync.nop(nofuse=True, hint="tail_wait")
            n.ins.sync_info = mybir.SyncInfo(on_wait=[w], on_update=[])
    nc.sync.drain()
    nc.all_engine_barrier()
    assert self.sems is not None
    popped = nc._tile_sem_poison_stack.pop()
    assert popped is self._sem_poison
    nc.clear_and_free_semaphores(list(self.sems.allocated().values()))
    nc.all_engine_barrier()


tile.TileContext._drain_and_barrier = _patched_drain_and_barrier


def _split_multi_waits(nc):
    """This walrus build supports at most one sync-wait per instruction.
    Move extra waits onto same-engine NoOps placed immediately before."""
    for fn in nc.m.functions:
        for blk in fn.blocks:
            insts = blk.instructions
            if not any(i.sync_info and len(i.sync_info.on_wait) > 1
                       for i in insts):
                continue
            out = []
            for ins in insts:
                si = ins.sync_info
                if si is not None and len(si.on_wait) > 1:
                    waits = list(si.on_wait)
                    for w in waits[:-1]:
                        nop = mybir.InstNoOp(
                            name=nc.get_next_instruction_name(),
                            sync_info=mybir.SyncInfo(on_wait=[w], on_update=[]),
                            bass_nofuse=True,
                            engine=ins.engine,
                        )
                        try:
                            nc.register_instruction(nop, overwrite=True)
                        except Exception:
                            pass
                        out.append(nop)
                    ins.sync_info = mybir.SyncInfo(
                        on_wait=[waits[-1]], on_update=list(si.on_update))
                out.append(ins)
            blk.instructions = out


EPS = 1e-5
RATES = (1, 6, 12, 18)
PADS = (2, 6, 12, 18)       # W padding per branch (r=1 padded to 2: even base)
B, C, H, W = 8, 512, 96, 96
CQ = C // 4                 # 128, one partition chunk per branch
PIX = H * W
STRIP = 16                  # rows per pipeline strip
SNT = STRIP * W             # 1536 px per strip
N_STRIPS = H // STRIP       # 6
GROUP = 4                   # r1 PE conv: rows per psum group (384 px <= 512)
CHUNK = 512                 # px per stage-2/3 psum bank chunk
F16 = mybir.dt.float16
F32 = mybir.dt.float32

_PROG_CACHE = {}

# Engine assignment knobs for the DVE-side branches (branch 0 is PE-only):
# ACT_TAPS: (i, t) whose tap product runs on ACT (scale-AP copy) not DVE.
# POOL_TAPS: per-branch taps accumulated on a Pool-side chain; the FIRST
#   listed tap must be clamp-free (dh=0, i.e. t in {3,5}) so the pool
#   accumulator covers the full strip before sub-range adds land on it.
ACT_TAPS = set()
POOL_TAPS = {2: (5, 0, 2), 3: (5, 0, 2)}
UNROLL = 8                # bodies per For_i iteration
STORE_ENGINE = "scalar"   # which engine's DMA ring carries output stores
                          # (gpsimd DMA hits "ISA wrong length" inside For_i)
CONSTS_IN_BODY = False    # reload weights every loop iteration (debug)


def _np_f16(a):
    return np.asarray(a, dtype=np.float32).astype(np.float16)


def _host_consts(wdw, bdw, gdw, bedw, mdw, vdw, ws, bs, gs, bes, ms, vs,
                 wo, bo, go, beo, mo, vo):
    """Fold BN constants into weights and build PE weight blocks."""
    f64 = np.float64
    # stage 1: y = relu(conv(x; w*s1) + b1)
    inv1 = np.asarray(gdw, f64) / np.sqrt(np.asarray(vdw, f64) + EPS)  # [4,128]
    b1 = (np.asarray(bdw, f64) - np.asarray(mdw, f64)) * inv1 + np.asarray(bedw, f64)
    wdw_s = np.asarray(wdw, f64) * inv1[:, :, None, None, None]        # [4,128,1,3,3]

    # stage 2: z_o = relu(sum_i Amix[o,i]*y_i + b2[o])
    invs = np.asarray(gs, f64) / np.sqrt(np.asarray(vs, f64) + EPS)    # [4]
    W4 = np.asarray(ws, f64)[:, :, 0, 0]                               # [o,i]
    Amix = W4 * invs[:, None]
    b2 = (np.asarray(bs, f64) - np.asarray(ms, f64)) * invs + np.asarray(bes, f64)

    # stage 3: out = relu(Wo' z + b3), Wo' = diag(s3) Wo
    inv3 = np.asarray(go, f64) / np.sqrt(np.asarray(vo, f64) + EPS)    # [512]
    Wo = np.asarray(wo, f64)[:, :, 0, 0]                               # [512,512]
    Wo_s = Wo * inv3[:, None]
    b3 = (np.asarray(bo, f64) - np.asarray(mo, f64)) * inv3 + np.asarray(beo, f64)

    consts = {}
    # r1 diag blocks: [128(k=c), 9(tap), 128(m=c)], s1-folded
    dw = np.zeros((CQ, 9, CQ), np.float32)
    for t in range(9):
        np.fill_diagonal(dw[:, t, :], wdw_s[0, :, 0, t // 3, t % 3].astype(np.float32))
    consts["dww"] = _np_f16(dw)

    # mix blocks: [128(k=c), 16(o*4+i), 128(m=c)] = Amix[o,i] * I
    mixw = np.zeros((CQ, 16, CQ), np.float32)
    for o in range(4):
        for i in range(4):
            np.fill_diagonal(mixw[:, o * 4 + i, :], np.float32(Amix[o, i]))
    consts["mixw"] = _np_f16(mixw)

    # stage-3 blocks: [128(k=c of z-chunk o), 16(m*4+o), 128(mc)]
    # z-chunk o, row c  <->  original z channel 4c+o
    s3w = np.zeros((CQ, 16, CQ), np.float32)
    for m in range(4):
        blk = Wo_s[128 * m:128 * (m + 1), :].astype(np.float32)  # [mc, 512]
        for o in range(4):
            s3w[:, m * 4 + o, :] = blk[:, o::4].T  # [c, mc]
    consts["s3w"] = _np_f16(s3w)

    # per-channel tap weights (s1-folded) for DVE/ACT: [128, 36], col i*9+t
    dws = np.zeros((CQ, 36), np.float32)
    for i in range(4):
        for t in range(9):
            dws[:, i * 9 + t] = wdw_s[i, :, 0, t // 3, t % 3].astype(np.float32)
    consts["dws"] = dws

    consts["b1"] = np.ascontiguousarray(np.asarray(b1, np.float32).T)  # [128,4]
    consts["b2"] = np.ascontiguousarray(
        np.broadcast_to(np.asarray(b2, np.float32)[None, :], (CQ, 4))).copy()
    consts["b3"] = np.ascontiguousarray(
        np.asarray(b3, np.float32).reshape(4, CQ).T)  # [128,4] col m
    return consts


def _tap_list(r, h0, h1):
    """Valid taps for output rows [h0, h1); center tap first (always full)."""
    taps = []
    for t in range(9):
        dh, dw = t // 3 - 1, t % 3 - 1
        lo = max(h0, -dh * r)
        hi = min(h1, H - dh * r)
        if lo < hi:
            taps.append((t, dh, dw, lo, hi))
    taps.sort(key=lambda e: (e[3] != h0 or e[4] != h1,))
    return taps


def _build_program(loop_n=None):
    nc = bass.Bass("TRN2", target_bir_lowering=False, debug=False, num_devices=8)

    xp_d = [nc.dram_tensor(f"xp{i}", [CQ, H, W + 2 * PADS[i]], F16,
                           kind="ExternalInput") for i in range(4)]
    dww_d = nc.dram_tensor("dww", [CQ, 9, CQ], F16, kind="ExternalInput")
    mixw_d = nc.dram_tensor("mixw", [CQ, 16, CQ], F16, kind="ExternalInput")
    s3w_d = nc.dram_tensor("s3w", [CQ, 16, CQ], F16, kind="ExternalInput")
    dws_d = nc.dram_tensor("dws", [CQ, 36], F32, kind="ExternalInput")
    b1_d = nc.dram_tensor("b1", [CQ, 4], F32, kind="ExternalInput")
    b2_d = nc.dram_tensor("b2", [CQ, 4], F32, kind="ExternalInput")
    b3_d = nc.dram_tensor("b3", [CQ, 4], F32, kind="ExternalInput")
    out_d = nc.dram_tensor("out", [C, PIX], F16, kind="ExternalOutput")

    with tile.TileContext(nc) as tc:
        with (
            tc.tile_pool(name="consts", bufs=1) as cpool,
            tc.tile_pool(name="xpad", bufs=1) as xpool,
            tc.tile_pool(name="ys", bufs=2) as ypool,
            tc.tile_pool(name="accs", bufs=2) as apool,
            tc.tile_pool(name="tmps", bufs=4) as tpool,
            tc.tile_pool(name="zs", bufs=2) as zpool,
            tc.tile_pool(name="outs", bufs=2) as opool,
            tc.tile_pool(name="ps1", bufs=2, space=bass.MemorySpace.PSUM) as ps1,
            tc.tile_pool(name="ps2", bufs=3, space=bass.MemorySpace.PSUM) as ps2,
            tc.tile_pool(name="ps3", bufs=3, space=bass.MemorySpace.PSUM) as ps3,
        ):
          # ---- constants to SBUF once, outside the timing loop
          dww = cpool.tile([CQ, 9, CQ], F16, tag="dww")
          nc.sync.dma_start(dww[:], dww_d[:])
          mixw = cpool.tile([CQ, 16, CQ], F16, tag="mixw")
          nc.sync.dma_start(mixw[:], mixw_d[:])
          s3w = cpool.tile([CQ, 16, CQ], F16, tag="s3w")
          nc.sync.dma_start(s3w[:], s3w_d[:])
          dws = cpool.tile([CQ, 36], F32, tag="dws")
          nc.sync.dma_start(dws[:], dws_d[:])
          b1 = cpool.tile([CQ, 4], F32, tag="b1")
          nc.sync.dma_start(b1[:], b1_d[:])
          b2 = cpool.tile([CQ, 4], F32, tag="b2")
          nc.sync.dma_start(b2[:], b2_d[:])
          b3 = cpool.tile([CQ, 4], F32, tag="b3")
          nc.sync.dma_start(b3[:], b3_d[:])

          xpad = []
          for i in range(4):
              wp = W + 2 * PADS[i]
              t = xpool.tile([CQ, H, wp], F16, tag=f"xpad{i}")
              xpad.append(t)

          def _body():
            # ---- padded x tiles (fp16, pre-padded on host)
            for i in range(4):
                for s in range(N_STRIPS):
                    nc.sync.dma_start(
                        xpad[i][:, s * STRIP:(s + 1) * STRIP, :],
                        xp_d[i][:, s * STRIP:(s + 1) * STRIP, :])

            def _r1_conv(s):
                # r1 branch on PE: diag matmuls per 4-row group
                h0 = s * STRIP
                pad0 = PADS[0]
                y0 = ypool.tile([CQ, SNT], F16, tag="y0")
                for g in range(STRIP // GROUP):
                    g0, g1 = h0 + g * GROUP, h0 + (g + 1) * GROUP
                    taps = _tap_list(1, g0, g1)
                    p1 = ps1.tile([CQ, CHUNK], F32, tag="p1")
                    for j, (t, dh, dw, lo, hi) in enumerate(taps):
                        rhs = xpad[0][:, lo + dh:hi + dh,
                                      pad0 + dw:pad0 + dw + W]
                        nc.tensor.matmul(
                            p1[:, (lo - g0) * W:(hi - g0) * W],
                            dww[:, t, :], rhs,
                            start=(j == 0), stop=(j == len(taps) - 1))
                    nc.scalar.activation(
                        y0[:, (g0 - h0) * W:(g1 - h0) * W],
                        p1[:, 0:GROUP * W],
                        mybir.ActivationFunctionType.Relu,
                        bias=b1[:, 0:1], scale=1.0)
                return y0

            # ---- per-strip pipeline; r1 conv runs one strip ahead so the
            # PE has independent work queued while stage 2 waits on DVE ys
            y0_next = _r1_conv(0)
            for s in range(N_STRIPS):
                h0, h1 = s * STRIP, (s + 1) * STRIP
                ys = [y0_next]

                # r6/r12/r18 branches on DVE (+ optional ACT/Pool offloads).
                # Two accumulator chains: `acc` on DVE, `accp` on Pool; the
                # pool chain is merged into acc at the end on DVE.
                for i in (1, 2, 3):
                    r, pad = RATES[i], PADS[i]
                    pool_taps = list(POOL_TAPS.get(i, ()))
                    acc = apool.tile([CQ, SNT], F16, tag=f"acc{i}")
                    if pool_taps:
                        accp = apool.tile([CQ, SNT], F16, tag=f"accp{i}",
                                          name=f"accp{i}")
                    else:
                        accp = None
                    dve_started = pool_started = False
                    for t, dh, dw, lo, hi in _tap_list(r, h0, h1):
                        xin = xpad[i][:, lo + dh * r:hi + dh * r,
                                      pad + dw * r:pad + dw * r + W]
                        wsc = dws[:, i * 9 + t:i * 9 + t + 1]
                        on_pool = t in pool_taps
                        tgt = accp if on_pool else acc
                        sub = tgt[:, (lo - h0) * W:(hi - h0) * W]
                        first = pool_started if on_pool else dve_started
                        if not first:
                            # init the chain with this tap's product
                            nc.vector.tensor_scalar_mul(sub, xin, wsc)
                            if on_pool:
                                pool_started = True
                            else:
                                dve_started = True
                            continue
                        tmp = tpool.tile([CQ, SNT], F16, tag="tmp")
                        tsub = tmp[:, 0:(hi - lo) * W]
                        if (i, t) in ACT_TAPS:
                            nc.scalar.activation(
                                tsub, xin,
                                mybir.ActivationFunctionType.Identity,
                                bias=0.0, scale=wsc)
                        else:
                            nc.vector.tensor_scalar_mul(tsub, xin, wsc)
                        eng = nc.gpsimd if on_pool else nc.vector
                        eng.tensor_tensor(
                            sub, sub, tsub, mybir.AluOpType.add)
                    if pool_taps:
                        nc.vector.tensor_tensor(
                            acc[:], acc[:], accp[:], mybir.AluOpType.add)
                    # epilogue: y = max(acc + b1, 0) at 4x on DVE
                    yb = ypool.tile([CQ, SNT], F16, tag=f"y{i}")
                    nc.vector.tensor_scalar(
                        yb[:], acc[:], b1[:, i:i + 1], 0.0,
                        mybir.AluOpType.add, mybir.AluOpType.max)
                    ys.append(yb)

                if s + 1 < N_STRIPS:
                    y0_next = _r1_conv(s + 1)

                # stage 2: mix on PE, 4 scaled-identity matmuls per (o, chunk)
                zs = []
                for o in range(4):
                    z = zpool.tile([CQ, SNT], F16, tag=f"z{o}")
                    for c in range(SNT // CHUNK):
                        p2 = ps2.tile([CQ, CHUNK], F32, tag="p2")
                        for i in range(4):
                            nc.tensor.matmul(
                                p2[:], mixw[:, o * 4 + i, :],
                                ys[i][:, c * CHUNK:(c + 1) * CHUNK],
                                start=(i == 0), stop=(i == 3))
                        nc.scalar.activation(
                            z[:, c * CHUNK:(c + 1) * CHUNK], p2[:],
                            mybir.ActivationFunctionType.Relu,
                            bias=b2[:, o:o + 1], scale=1.0)
                    zs.append(z)

                # stage 3: dense 512x512 on PE
                for m in range(4):
                    ot = opool.tile([CQ, SNT], F16, tag="ot")
                    for c in range(SNT // CHUNK):
                        p3 = ps3.tile([CQ, CHUNK], F32, tag="p3")
                        for o in range(4):
                            nc.tensor.matmul(
                                p3[:], s3w[:, m * 4 + o, :],
                                zs[o][:, c * CHUNK:(c + 1) * CHUNK],
                                start=(o == 0), stop=(o == 3))
                        nc.scalar.activation(
                            ot[:, c * CHUNK:(c + 1) * CHUNK], p3[:],
                            mybir.ActivationFunctionType.Relu,
                            bias=b3[:, m:m + 1], scale=1.0)
                    # stores on a separate DMA ring: keeps the sync ring
                    # loads-only so iter k+1 loads don't queue behind stores
                    store_eng = getattr(nc, STORE_ENGINE)
                    store_eng.dma_start(
                        out_d[CQ * m:CQ * (m + 1), h0 * W:h0 * W + SNT], ot[:])

          if loop_n:
              # For_i places an all-engine barrier + semaphore reset on
              # every iteration, serializing the pipeline at the back edge.
              # Unroll several bodies per iteration so the barrier cost
              # amortizes and bodies overlap through pool double-buffering.
              u = math.gcd(loop_n, UNROLL)
              with tc.For_i(0, loop_n // u, 1):
                  for _ in range(u):
                      _body()
          else:
              _body()
    _split_multi_waits(nc)
    return nc


def _get_program(loop_n=None):
    key = ("nc", loop_n)
    if key not in _PROG_CACHE:
        _PROG_CACHE[key] = _build_program(loop_n)
    return _PROG_CACHE[key]


def _in_maps(x, consts):
    x16 = np.asarray(x, np.float32).astype(np.float16).reshape(B, 4, CQ, H, W)
    maps = []
    for b in range(B):
        m = dict(consts)
        for i in range(4):
            p = PADS[i]
            m[f"xp{i}"] = np.ascontiguousarray(
                np.pad(x16[b, i], ((0, 0), (0, 0), (p, p))))
        maps.append(m)
    return maps


def run(x, consts, trace=False, loop_n=None):
    nc = _get_program(loop_n)
    res = run_bass_kernel_spmd(nc, _in_maps(x, consts), list(range(B)),
                               trace=trace)
    out = np.stack([res.results[b]["out"].reshape(C, H, W) for b in range(B)])
    return out.astype(np.float32), res


def kernel(x, wdw, bdw, gdw, bedw, mdw, vdw, ws, bs, gs, bes, ms, vs,
           wo, bo, go, beo, mo, vo):
    consts = _host_consts(wdw, bdw, gdw, bedw, mdw, vdw, ws, bs, gs, bes,
                          ms, vs, wo, bo, go, beo, mo, vo)
    out, _ = run(x, consts, trace=False)
    return out
